# revision 55
# baseline (speedup 1.0000x reference)
"""Trainium2 Bass kernel for Mistral-style attention with an INVERTED band mask.

Reference semantics (S=2048, E=4096, H=32, KV=8, D=128, WINDOW=1024):
  q/k/v projections -> RoPE(q,k) -> GQA attention where positions with
  |i-j| < 1024 are masked OUT (attend only to far positions) -> softmax ->
  out projection.

Sharding (8 cores, tensor-parallel by GQA group):
  core c owns KV head c and Q heads 4c..4c+3. Column-parallel QKV,
  row-parallel O projection; the 8 fp16 partial outputs are summed on host.

Schedule (v3: baseline 572us -> 373us -> 365us -> ~350us, PE ~92% busy):
  - DMA: the queues round-robin per DESCRIPTOR, so a transfer's bandwidth
    share is proportional to its per-partition contiguous run length. ALL
    phase-1 input streams move as 2KB-per-partition pieces (2 e-tiles):
    hid flows through an 8-tile rotation buffer in global need order,
    where piece k's DMA is emitted just before piece k-8's matmuls so the
    WAR dependency self-throttles prefetch to 2MB ahead; wq rides sync,
    wkv gpsimd. First matmul ~12us; phase 1 then runs gap-free at the
    warm 216ns/MM stream rate.
  - Phase 1 projects chunks [3,0,1] only. Chunk 2's Q and K/V projections
    are deferred into the attention phase as PE "feeds": pure attention
    is ACT/DVE-bound (exp+etsum ~1.3ns/col vs PE 0.83ns/col), so
    attention chunks would stall the PE without interleaved projection /
    O-projection matmuls. Sections: S1 att3 x c2-Q-proj | S2 att1 x
    oproj3-half | S3 att2 x oproj3 x c2-KV-proj | S4 att0 x oproj1 | S5
    oproj2+oproj0. Feeds pop between a score matmul and its AV matmul,
    hiding the exp latency; PSUM budgets to exactly 8 banks per section
    (the denominator shares the score tile's ring).
  - Per-chunk qT/kT/v/attn tiles (no false whole-tile deps); per-head
    rope drains split across ACT/DVE; deferred rope finishes ride the
    section head-hooks.
  - Softmax denominator via an all-ones stationary matmul over the DVE
    etsum accumulation (widest entry first -> plain copy, no memset);
    normalize is approx-reciprocal + multiply on DVE.
  - O-projection output rows DMA out in quarters to shorten the tail.
  - HW exec ~350us vs a ~311us tensor-engine column-count floor; fp8
    (DoubleRow) was measured to break the 2e-2 accuracy gate (median rel
    err 4e-2 with a single fp8 GEMM stage), so everything stays fp16.
"""

import math
from contextlib import ExitStack

import numpy as np
import ml_dtypes

import concourse.bass as bass
import concourse.mybir as mybir
import concourse.tile as tile
from concourse import bacc
from concourse.bass_utils import run_bass_kernel_spmd

P = 128
S = 2048
E = 4096
D = 128
HPC = 4          # q heads per core
NE = E // P      # 32 e-tiles
NSCH = 4         # s-chunks of 512
SCH = S // NSCH  # 512
NST = S // P     # 16 s-tiles
NEO = 8          # output e-chunks of 512
GE = 8           # e-tiles per hid DMA group
NG = NE // GE    # 4 groups
SCALE = 1.0 / math.sqrt(D)
F16 = mybir.dt.float16
F32 = mybir.dt.float32
BF16 = mybir.dt.bfloat16


def _allowed_tiles(c):
    """For s-chunk c (query blocks bi=4c..4c+3), list (bj, lo, hi, mask, mpos):
    key tile bj is needed for query sub-tiles [lo, hi) (chunk-relative);
    mask in {None,'low','up'} applied at chunk-relative position mpos."""
    out = []
    bis = range(4 * c, 4 * c + 4)
    for bj in range(NST):
        ok = [bi for bi in bis if abs(bi - bj) >= 8]
        if not ok:
            continue
        lo = min(ok) - 4 * c
        hi = max(ok) + 1 - 4 * c
        assert ok == list(range(lo + 4 * c, hi + 4 * c)), (c, bj, ok)
        mask, mpos = None, 0
        if bj - 8 in ok:
            mask, mpos = "low", bj - 8 - 4 * c
        elif bj + 8 in ok:
            mask, mpos = "up", bj + 8 - 4 * c
        out.append((bj, lo, hi, mask, mpos))
    # widest entry first: the first etsum op can then be a plain copy
    # covering the whole chunk (no per-head memset needed)
    out.sort(key=lambda t: t[1] - t[2])
    assert out[0][1] == 0 and out[0][2] == 4, (c, out[0])
    return out


def build_nc(debug=False):
    nc = bacc.Bacc("TRN2", target_bir_lowering=False, debug=False)
    # host-relaid tensors: partition-major, contiguous per partition
    hidw = nc.dram_tensor("hidw", (P, NSCH * NG, GE * SCH), F16,
                          kind="ExternalInput")
    wqw = nc.dram_tensor("wqw", (P, NG, GE * HPC * D), F16,
                         kind="ExternalInput")
    wkvw = nc.dram_tensor("wkvw", (P, NG, GE * 2 * D), F16,
                          kind="ExternalInput")
    wow = nc.dram_tensor("wow", (P, HPC * E), F16, kind="ExternalInput")
    cosT = nc.dram_tensor("cosT", (D, S), F16, kind="ExternalInput")
    sinT = nc.dram_tensor("sinT", (D, S), F16, kind="ExternalInput")
    mlow = nc.dram_tensor("mlow", (P, P), BF16, kind="ExternalInput")
    mup = nc.dram_tensor("mup", (P, P), BF16, kind="ExternalInput")
    outd = nc.dram_tensor("out", (S, E), F16, kind="ExternalOutput")

    with tile.TileContext(nc) as tc, ExitStack() as ctx:
        const = ctx.enter_context(tc.tile_pool(name="const", bufs=1))

        # DMA granularity: the queues round-robin per DESCRIPTOR, so a
        # transfer's share of HBM bandwidth is proportional to its
        # per-partition contiguous run length. Everything phase-1 moves in
        # 2KB-per-partition pieces (2 e-tiles) so delivery order tracks the
        # MM stream's need order; any bulk tile would hog 4-8x its share
        # and starve the others.
        wq_f = [const.tile([P, 2 * HPC * D], F16, name=f"wqf{e}")
                for e in range(NE // 2)]
        wkv_f = [const.tile([P, 4 * 2 * D], F16, name=f"wkvf{e}")
                 for e in range(NE // 4)]
        wo_sb = const.tile([P, HPC, E], F16)
        cos_sb = const.tile([P, S], F16)
        sin_sb = const.tile([P, S], F16)
        ml_sb = const.tile([P, P], BF16)
        mu_sb = const.tile([P, P], BF16)
        ones_sb = const.tile([P, P], BF16)

        # per-chunk roped Q^T / K^T / V / attention-output tiles (separate
        # tiles so tile-granular dep tracking can't create false waits)
        qT_c = [const.tile([P, HPC, SCH], F16, name=f"qT{c}")
                for c in range(NSCH)]
        kT_c = [const.tile([P, SCH], F16, name=f"kT{c}") for c in range(NSCH)]
        v_c = [const.tile([P, 4, D], F16, name=f"v{c}") for c in range(NSCH)]
        attn_c = [const.tile([P, HPC, SCH], F16, name=f"at{c}")
                  for c in range(NSCH)]

        # hid streams through an 8-tile rotation: piece k's DMA carries a
        # WAR dependency on piece k-8's matmuls, so issue (and transfer)
        # self-throttles to 8 pieces (2MB) ahead of consumption. All hid
        # rides one queue in global need order.
        NPC = NE // 2                       # 16 pieces per chunk
        hidb = [const.tile([P, 2 * SCH], F16, name=f"hidb{i}")
                for i in range(8)]

        rp = ctx.enter_context(tc.tile_pool(name="rope", bufs=2))

        # Phase 1 projects chunks [3,0,1]; chunk 2's Q and K/V projections
        # are deferred into the attention phase as PE "feeds": a pure
        # attention window is ACT/DVE-bound (exp+etsum ~1.3ns/col vs PE
        # 0.83ns/col), so the first attention chunk would stall the PE
        # unless projection matmuls are interleaved into it.
        ph1_order = [3, 0, 1]
        c_last = 2
        c_first = ph1_order[0]
        # hid piece stream: phase-1 chunks, then c2 twice (once for its
        # Q projection in S1, re-fetched for its K/V projection in S3)
        HID_SCHED = ph1_order + [c_last, c_last]

        # ---- startup DMA: fine-grained first parcel -----------------------
        # e-tile e of the first chunk needs hid[e] (128KB) + wq[e] (128KB)
        # + wk/wv[e] (32KB each); with per-e-tile tiles on four queues the
        # first MM starts as soon as ~0.32MB lands (~1.5us of transfer)
        # instead of waiting for 2.6MB of monolithic tiles.
        nc.gpsimd.memset(ones_sb[:], 1.0)
        for e2 in range(NPC):
            g, eq = divmod(e2, GE // 2)
            nc.sync.dma_start(
                wq_f[e2][:],
                wqw[:, g, eq * 2 * HPC * D:(eq + 1) * 2 * HPC * D])
        nc.sync.dma_start(cos_sb[:], cosT[:])
        nc.sync.dma_start(sin_sb[:], sinT[:])
        nc.sync.dma_start(ml_sb[:], mlow[:])
        nc.sync.dma_start(mu_sb[:], mup[:])
        # hid pieces stream through the rotation in global need order on one
        # queue. Piece k's DMA is EMITTED just before piece k-8's matmuls
        # (in hid_dma below) so program-order WAR tracking gates transfer to
        # 8 pieces (2MB) ahead of consumption. Startup: prefetch 8 pieces,
        # wkv interleaved.
        def hid_dma(pc, eng=None):
            ci, e2 = divmod(pc, NPC)
            c = HID_SCHED[ci]
            g, ee2 = divmod(e2, GE // 2)
            (eng or nc.gpsimd).dma_start(
                hidb[pc % 8][:],
                hidw[:, c * NG + g, ee2 * 2 * SCH:(ee2 + 1) * 2 * SCH])
            if pc % 2 == 0 and pc < 16:
                g4, ek = divmod(pc // 2, GE // 4)
                nc.gpsimd.dma_start(
                    wkv_f[pc // 2][:],
                    wkvw[:, g4, ek * 8 * D:(ek + 1) * 8 * D])

        # prefetch alternates gpsimd/scalar so the first pieces land in
        # parallel instead of serializing behind each other
        for pc in range(8):
            hid_dma(pc, eng=nc.scalar if pc % 2 else None)

        def rope_k_rest(kraw, c):
            csl = slice(c * SCH, (c + 1) * SCH)
            krot = rp.tile([P, SCH], F16, tag="krot", bufs=2)
            nc.sync.dma_start(krot[0:64, :], kraw[64:128, :])
            nc.sync.dma_start(krot[64:128, :], kraw[0:64, :])
            nc.vector.tensor_tensor(
                kraw[:], kraw[:], cos_sb[:, csl], mybir.AluOpType.mult)
            nc.vector.tensor_tensor(
                krot[:], krot[:], sin_sb[:, csl], mybir.AluOpType.mult)
            nc.vector.tensor_tensor(
                kT_c[c][:], kraw[:], krot[:], mybir.AluOpType.add)

        def rope_q_drain(psq1, drain_eng=None):
            """copy head PSUM to SBUF + start the rotate-half swap DMA"""
            qraw = rp.tile([P, SCH], F16, tag="qraw", bufs=4)
            if drain_eng == "vector":
                nc.vector.tensor_copy(qraw[:], psq1)
            else:
                nc.scalar.copy(qraw[:], psq1)
            qrot = rp.tile([P, SCH], F16, tag="qrot", bufs=4)
            nc.sync.dma_start(qrot[0:64, :], qraw[64:128, :])
            nc.sync.dma_start(qrot[64:128, :], qraw[0:64, :])
            return qraw, qrot

        def rope_q_finish(qq, h, c):
            qraw, qrot = qq
            csl = slice(c * SCH, (c + 1) * SCH)
            nc.vector.tensor_tensor(
                qraw[:], qraw[:], cos_sb[:, csl], mybir.AluOpType.mult)
            nc.vector.tensor_tensor(
                qrot[:], qrot[:], sin_sb[:, csl], mybir.AluOpType.mult)
            nc.vector.tensor_tensor(
                qT_c[c][:, h, :], qraw[:], qrot[:], mybir.AluOpType.add)

        # ---- Phase 1: QKV projections (+RoPE) ----
        with tc.tile_pool(name="p1psum", bufs=1, space="PSUM") as p1:
            for ci, c in enumerate(ph1_order):
                psqA = p1.tile([P, 2, SCH], F32, tag="psq2", bufs=3)
                psqB = p1.tile([P, 2, SCH], F32, tag="psq2", bufs=3)
                psk = p1.tile([P, SCH], F32, tag="psk", bufs=1)
                psv = p1.tile([P, SCH], F32, tag="psv", bufs=1)
                for e in range(NE):
                    pc = ci * NPC + e // 2
                    rhs = hidb[pc % 8][:, (e % 2) * SCH:(e % 2 + 1) * SCH]
                    st = (e == 0)
                    sp = (e == NE - 1)
                    qo = (e % 2) * HPC * D
                    kvo = (e % 4) * 2 * D
                    for h in range(HPC):
                        dst = psqA[:, h, :] if h < 2 else psqB[:, h - 2, :]
                        nc.tensor.matmul(
                            dst, wq_f[e // 2][:, qo + h * D:qo + (h + 1) * D],
                            rhs, start=st, stop=sp)
                    nc.tensor.matmul(
                        psk[:], wkv_f[e // 4][:, kvo:kvo + D], rhs,
                        start=st, stop=sp)
                    nc.tensor.matmul(
                        psv[:], wkv_f[e // 4][:, kvo + D:kvo + 2 * D], rhs,
                        start=st, stop=sp)
                    # refill the rotation slot this piece just vacated
                    if e % 2 == 1:
                        hid_dma(pc + 8)
                # Drains split across ACT (h0,h2,kraw) and DVE (h1,h3,
                # vstage) so each psq tile's two banks free in ~0.7us;
                # the last chunk drains h2/h3 first (their banks become
                # the attention score tiles) and defers its q-rope
                # finishes into the S1 attention stream.
                last = (ci == len(ph1_order) - 1)
                kraw = rp.tile([P, SCH], F16, tag="kraw", bufs=2)
                vstage = rp.tile([P, SCH], F16, tag="vstage", bufs=2)
                jorder = (2, 3, 0, 1) if last else (0, 1, 2, 3)
                qq = {}
                for j in jorder:
                    ps = psqB if j >= 2 else psqA
                    qq[j] = rope_q_drain(ps[:, j % 2, :],
                                         drain_eng="vector" if j % 2 else None)
                nc.scalar.copy(kraw[:], psk[:])
                nc.vector.tensor_copy(vstage[:], psv[:])
                rope_k_rest(kraw, c)
                if last:
                    q1_pend = qq
                else:
                    for j in jorder:
                        rope_q_finish(qq[j], j, c)
                nc.sync.dma_start_transpose(v_c[c][:], vstage[:])

        # ---- S1..S5: attention with projection / O-projection feeds -------
        ep = ctx.enter_context(tc.tile_pool(name="expp", bufs=3))
        np_pool = ctx.enter_context(tc.tile_pool(name="normp", bufs=2))
        osp = ctx.enter_context(tc.tile_pool(name="ostage", bufs=2))

        def attn_chunk(c, pool, feeds=None, fpe=1, pre=0, head_hook=None):
            """One chunk of attention. feeds: closures each emitting ~1us
            of independent PE work, popped between a score matmul and its
            AV matmul (covers the exp latency and soaks up the window's
            spare PE capacity)."""
            entries = _allowed_tiles(c)
            nblk = len(entries)
            feeds = list(feeds or [])
            for _ in range(pre):
                feeds.pop(0)()
            for h in range(HPC):
                psa = pool.tile([P, SCH], F32, tag="psa", bufs=2)
                etsum = ep.tile([P, SCH], BF16, tag="etsum", bufs=2)
                for idx, (bj, lo, hi, mask, mpos) in enumerate(entries):
                    n = (hi - lo) * P
                    pss = pool.tile([P, SCH], F32, tag="pss", bufs=2)
                    nc.tensor.matmul(
                        pss[:, :n],
                        kT_c[bj // 4][:, (bj % 4) * P:(bj % 4 + 1) * P],
                        qT_c[c][:, h, lo * P:hi * P],
                        start=True, stop=True, skip_group_check=True)
                    et = ep.tile([P, SCH], BF16, tag="et")
                    nc.scalar.activation(
                        et[:, :n], pss[:, :n],
                        mybir.ActivationFunctionType.Exp, scale=SCALE)
                    if mask is not None:
                        msb = ml_sb if mask == "low" else mu_sb
                        nc.vector.tensor_tensor(
                            et[:, (mpos - lo) * P:(mpos - lo + 1) * P],
                            et[:, (mpos - lo) * P:(mpos - lo + 1) * P],
                            msb[:], mybir.AluOpType.mult)
                    for _ in range(min(fpe, len(feeds))):
                        feeds.pop(0)()
                    nc.tensor.matmul(
                        psa[:, lo * P:hi * P],
                        v_c[bj // 4][:, bj % 4, :], et[:, :n],
                        start=(idx == 0), stop=(idx == nblk - 1),
                        skip_group_check=True)
                    # off the AV critical path: only the one denominator
                    # matmul at the end of the head waits on this chain
                    if idx == 0:
                        nc.vector.tensor_copy(etsum[:], et[:, :n])
                    else:
                        nc.vector.tensor_tensor(
                            etsum[:, lo * P:hi * P],
                            etsum[:, lo * P:hi * P],
                            et[:, :n], mybir.AluOpType.add)
                psd = pool.tile([P, SCH], F32, tag="pss", bufs=2)
                nc.tensor.matmul(
                    psd[:], ones_sb[:], etsum[:], start=True, stop=True,
                    skip_group_check=True)
                rcp = np_pool.tile([P, SCH], F32, tag="rcp")
                nc.vector.reciprocal_approx_fast(rcp[:], psd[:])
                nc.vector.tensor_tensor(
                    attn_c[c][:, h, :], psa[:], rcp[:],
                    mybir.AluOpType.mult)
                if head_hook:
                    head_hook(h)
            assert not feeds, (c, len(feeds))

        def oproj_feeds(c, pool):
            """32 closures, each emitting one [128,512] O-projection psum
            group (4 matmuls + drain + output quarter-DMA)."""
            orow_state = {}
            fs = []
            for j in range(4):
                for eo in range(NEO):
                    def f(j=j, eo=eo):
                        st = 4 * c + j
                        if eo == 0:
                            orow_state[j] = osp.tile([P, E], F16, tag="orow",
                                                     name=f"orow{c}_{j}")
                        orow = orow_state[j]
                        pso = pool.tile([P, SCH], F32, tag="pso", bufs=2)
                        for h in range(HPC):
                            nc.tensor.matmul(
                                pso[:],
                                attn_c[c][:, h, j * P:(j + 1) * P],
                                wo_sb[:, h, eo * SCH:(eo + 1) * SCH],
                                start=(h == 0), stop=(h == HPC - 1),
                                skip_group_check=True)
                        if eo % 2 == 0:
                            nc.vector.tensor_copy(
                                orow[:, eo * SCH:(eo + 1) * SCH], pso[:])
                        else:
                            nc.scalar.copy(
                                orow[:, eo * SCH:(eo + 1) * SCH], pso[:])
                            qo = eo // 2
                            nc.sync.dma_start(
                                outd[st * P:(st + 1) * P,
                                     qo * 2 * SCH:(qo + 1) * 2 * SCH],
                                orow[:, qo * 2 * SCH:(qo + 1) * 2 * SCH])
                    fs.append(f)
            return fs

        def q_feed_list(psqA2, psqB2):
            """32 closures: c2's Q projection, one e-tile (4 matmuls) each,
            consuming hid pieces 48-63 and refilling the rotation."""
            fs = []
            for e in range(NE):
                def f(e=e):
                    pc = 3 * NPC + e // 2
                    rhs = hidb[pc % 8][:, (e % 2) * SCH:(e % 2 + 1) * SCH]
                    qo = (e % 2) * HPC * D
                    st = (e == 0)
                    sp = (e == NE - 1)
                    for h in range(HPC):
                        dst = psqA2[:, h, :] if h < 2 else psqB2[:, h - 2, :]
                        nc.tensor.matmul(
                            dst, wq_f[e // 2][:, qo + h * D:qo + (h + 1) * D],
                            rhs, start=st, stop=sp, skip_group_check=True)
                    if e % 2 == 1:
                        hid_dma(pc + 8)
                fs.append(f)
            return fs

        def kv_feed_list(psk2, psv2):
            """16 closures: c2's K/V projection, two e-tiles (4 matmuls)
            each, consuming re-fetched hid pieces 64-79."""
            fs = []
            for p in range(NE // 2):
                def f(p=p):
                    for e in (2 * p, 2 * p + 1):
                        pc = 4 * NPC + e // 2
                        rhs = hidb[pc % 8][:, (e % 2) * SCH:(e % 2 + 1) * SCH]
                        kvo = (e % 4) * 2 * D
                        st = (e == 0)
                        sp = (e == NE - 1)
                        nc.tensor.matmul(
                            psk2[:], wkv_f[e // 4][:, kvo:kvo + D], rhs,
                            start=st, stop=sp, skip_group_check=True)
                        nc.tensor.matmul(
                            psv2[:], wkv_f[e // 4][:, kvo + D:kvo + 2 * D],
                            rhs, start=st, stop=sp, skip_group_check=True)
                        if e % 2 == 1 and pc + 8 < 5 * NPC:
                            hid_dma(pc + 8)
                fs.append(f)
            return fs

        # S1: att3 with c2's Q projection as feeds. Hooks: finish c1's
        # q-rope (one head per hook, keeping DVE clear early) + stream one
        # wo piece per hook; drain c2's Q PSUM at the last head.
        with tc.tile_pool(name="w1psum", bufs=1, space="PSUM") as w1:
            psqA2 = w1.tile([P, 2, SCH], F32, tag="qa")
            psqB2 = w1.tile([P, 2, SCH], F32, tag="qb")
            q2_pend = {}

            def hook1(h):
                rope_q_finish(q1_pend[h], h, ph1_order[-1])
                nc.gpsimd.dma_start(wo_sb[:, h, :], wow[:, h * E:(h + 1) * E])
                if h == 3:
                    for j in (2, 3, 0, 1):
                        ps = psqB2 if j >= 2 else psqA2
                        q2_pend[j] = rope_q_drain(
                            ps[:, j % 2, :],
                            drain_eng="vector" if j % 2 else None)

            attn_chunk(3, w1, feeds=q_feed_list(psqA2, psqB2),
                       head_hook=hook1)

        # S2..S5: one PSUM pool (psk/psv for c2's K/V + score/AV/O tiles)
        with tc.tile_pool(name="w2psum", bufs=1, space="PSUM") as w2:
            psk2 = w2.tile([P, SCH], F32, tag="psk")
            psv2 = w2.tile([P, SCH], F32, tag="psv")
            of3 = oproj_feeds(3, w2)

            def hook2(h):
                j = (2, 3, 0, 1)[h]
                rope_q_finish(q2_pend[j], j, c_last)

            # S2: att1 with the first half of oproj3 as feeds
            attn_chunk(1, w2, feeds=of3[:16], head_hook=hook2)

            # S3: att2 with the rest of oproj3 + c2's K/V projection
            kvf = kv_feed_list(psk2, psv2)
            feeds3 = [x for pair in zip(of3[16:], kvf) for x in pair]

            def hook3(h):
                if h == 3:
                    kraw2 = rp.tile([P, SCH], F16, tag="kraw", bufs=2)
                    nc.scalar.copy(kraw2[:], psk2[:])
                    vst2 = rp.tile([P, SCH], F16, tag="vstage", bufs=2)
                    nc.vector.tensor_copy(vst2[:], psv2[:])
                    rope_k_rest(kraw2, c_last)
                    nc.sync.dma_start_transpose(v_c[c_last][:], vst2[:])

            attn_chunk(2, w2, feeds=feeds3, fpe=2, head_hook=hook3)

            # S4: att0 with oproj1 as feeds (pre=2 covers c2's k-rope tail)
            attn_chunk(0, w2, feeds=oproj_feeds(1, w2), pre=2)

            # S5: remaining O-projections
            for f in oproj_feeds(2, w2):
                f()
            for f in oproj_feeds(0, w2):
                f()
    nc.compile()
    return nc


_NC_CACHE = {}


def get_nc():
    if "nc" not in _NC_CACHE:
        _NC_CACHE["nc"] = build_nc()
    return _NC_CACHE["nc"]


def make_in_maps(hidden_states, Wq, Wk, Wv, Wo):
    hid = np.asarray(hidden_states).reshape(S, E)
    hidT16 = np.ascontiguousarray(hid.T).astype(np.float16)   # [E, S]
    # [p, c, g, ee, s'] contiguous per partition
    hidw = (hidT16.reshape(NG, GE, P, NSCH, SCH)
            .transpose(2, 3, 0, 1, 4)
            .reshape(P, NSCH * NG, GE * SCH))
    hidw = np.ascontiguousarray(hidw)

    inv = 1.0 / (10000.0 ** (np.arange(0, D, 2, dtype=np.float64) / D))
    t = np.arange(S, dtype=np.float64)
    fr = np.outer(t, inv)                      # [S, 64]
    emb = np.concatenate([fr, fr], axis=1)     # [S, 128]
    cosT = np.ascontiguousarray(np.cos(emb).T).astype(np.float16)
    sinT = np.sin(emb).T.copy()
    sinT[:64] *= -1.0                          # rotate_half sign fold
    sinT = np.ascontiguousarray(sinT).astype(np.float16)

    jj = np.arange(P)[:, None]
    ii = np.arange(P)[None, :]
    mlow = (jj >= ii).astype(ml_dtypes.bfloat16)   # block bj-bi=8: j-i>=1024
    mup = (ii >= jj).astype(ml_dtypes.bfloat16)    # block bi-bj=8: i-j>=1024

    def wlayout(w, inner):
        # w [E_in, cols] -> [p, g, ee, cols] contiguous per partition
        arr = np.ascontiguousarray(w.T).astype(np.float16)   # [E_in, cols]
        return np.ascontiguousarray(
            arr.reshape(NG, GE, P, inner).transpose(2, 0, 1, 3)
            .reshape(P, NG, GE * inner))

    in_maps = []
    for c in range(8):
        qsl = slice(c * 512, (c + 1) * 512)
        ksl = slice(c * 128, (c + 1) * 128)
        wo_c = np.ascontiguousarray(Wo[:, qsl].T).astype(np.float16)  # [512, E]
        wow = np.ascontiguousarray(
            wo_c.reshape(HPC, P, E).transpose(1, 0, 2).reshape(P, HPC * E))
        wkvw = np.ascontiguousarray(np.concatenate(
            [wlayout(Wk[ksl], D).reshape(P, NG, GE, D),
             wlayout(Wv[ksl], D).reshape(P, NG, GE, D)],
            axis=3).reshape(P, NG, GE * 2 * D))
        in_maps.append({
            "hidw": hidw,
            "wqw": wlayout(Wq[qsl], HPC * D),
            "wkvw": wkvw,
            "wow": wow,
            "cosT": cosT,
            "sinT": sinT,
            "mlow": mlow,
            "mup": mup,
        })
    return in_maps


def run(in_maps, **kwargs):
    nc = get_nc()
    return run_bass_kernel_spmd(nc, in_maps, core_ids=list(range(8)), **kwargs)


def kernel(hidden_states, Wq, Wk, Wv, Wo):
    in_maps = make_in_maps(hidden_states, Wq, Wk, Wv, Wo)
    res = run(in_maps)
    out = np.zeros((S, E), dtype=np.float32)
    for r in res.results:
        out += r["out"].astype(np.float32)
    return out.reshape(1, S, E)
